# revision 10
# baseline (speedup 1.0000x reference)
"""Trainium2 Bass kernel for nn_AttentionCell (Bahdanau attention + LSTM cell).

Full shapes: B=256, T=256, D_ENC=512, H=512, NUM_CLASSES=96.
Sharding: data-parallel over batch across 8 NeuronCores (32 rows each),
all parameters replicated.

Per-core plan:
  - load batch_H[b] naturally [T, D] as float32r (SWDGE cast-DMA rounds) ->
    PE-transpose to bHT [D, T] tiles (f32r transpose = 1.5 cyc/row)
  - G.T[h, t] = Wi2h.T @ bH.T via PE float32r matmuls (1 cyc/row for N>=256,
    vs 4 cyc/row for plain fp32; ~1e-4 rounding)
  - tanh(G.T + h_projT[:, b]) fused on ScalarE (bias is per-partition in this
    orientation), output rounded to f32r
  - e[1, t] = Wscore.T @ tanhG.T on PE
  - softmax over t on partition 0 (DVE max(negate) -> ACT exp with accum_out
    -> DVE reciprocal -> DVE tensor_scalar mul), all plain fp32
  - alphaT column via tiny fp32 PE matmul against ones; context accumulates
    into a single [32, 512] PSUM tile using zero-padded alphaT columns as the
    stationary operand (f32r)
  - LSTM: z = x @ Wk + prev_h @ Uk + bk on PE (f32r) with host-pretransposed
    prev_h.T / onehots.T; gates on ScalarE/VectorE in fp32
"""

import os
import sys
from contextlib import ExitStack

import numpy as np

sys.path.insert(0, "/opt/trn_rl_repo")

import concourse.bass as bass  # noqa: E402,F401
import concourse.tile as tile  # noqa: E402
from concourse import bacc, mybir  # noqa: E402
from concourse.masks import make_identity  # noqa: E402

F32 = mybir.dt.float32
F32R = mybir.dt.float32r
AF = mybir.ActivationFunctionType
ALU = mybir.AluOpType
AX = mybir.AxisListType

# Problem constants (hardcoded; kernel.py must be self-contained)
B, T, D, H, NCLS = 256, 256, 512, 512, 96
NCORES = 8
NB = B // NCORES           # 32 batch rows per core
ZD = 4 * H                 # 2048 LSTM gate width

LAST_RESULT = None         # stashes BassKernelResults for test harness


def build_bass():
    nc = bacc.Bacc("TRN2")

    # ---------------- DRAM I/O (per-core shard shapes) ----------------
    d_batch_H = nc.dram_tensor("batch_H", [NB, T, D], F32, kind="ExternalInput")
    d_prev_hT = nc.dram_tensor("prev_hT", [H, NB], F32, kind="ExternalInput")
    d_prev_c = nc.dram_tensor("prev_c", [NB, H], F32, kind="ExternalInput")
    d_ohT = nc.dram_tensor("ohT", [NCLS, NB], F32, kind="ExternalInput")
    d_Wi2h = nc.dram_tensor("Wi2h", [D, H], F32, kind="ExternalInput")
    d_Wh2h = nc.dram_tensor("Wh2h", [H, H], F32, kind="ExternalInput")
    d_bh2h = nc.dram_tensor("bh2h", [1, H], F32, kind="ExternalInput")
    d_WscT = nc.dram_tensor("WscT", [128, 4], F32, kind="ExternalInput")
    d_Wk_main = nc.dram_tensor("Wk_main", [D, ZD], F32, kind="ExternalInput")
    d_Wk_tail = nc.dram_tensor("Wk_tail", [NCLS, ZD], F32, kind="ExternalInput")
    d_Uk = nc.dram_tensor("Uk", [H, ZD], F32, kind="ExternalInput")
    d_bk = nc.dram_tensor("bk", [1, ZD], F32, kind="ExternalInput")

    d_h_new = nc.dram_tensor("h_new", [NB, H], F32, kind="ExternalOutput")
    d_c_new = nc.dram_tensor("c_new", [NB, H], F32, kind="ExternalOutput")
    d_alpha = nc.dram_tensor("alpha", [NB, T], F32, kind="ExternalOutput")

    bh = d_batch_H.ap()

    with tile.TileContext(nc) as tc, ExitStack() as ctx:
        singles = ctx.enter_context(tc.tile_pool(name="singles", bufs=1))
        nat_pool = ctx.enter_context(tc.tile_pool(name="nat", bufs=2))
        bht_pool = ctx.enter_context(tc.tile_pool(name="bht", bufs=2))
        tg_pool = ctx.enter_context(tc.tile_pool(name="tg", bufs=2))
        sm_pool = ctx.enter_context(tc.tile_pool(name="sm", bufs=3))
        ps_tp = ctx.enter_context(tc.tile_pool(name="ps_tp", bufs=1, space="PSUM"))
        ps_g = ctx.enter_context(tc.tile_pool(name="ps_g", bufs=1, space="PSUM"))
        ps_ev = ctx.enter_context(tc.tile_pool(name="ps_ev", bufs=2, space="PSUM"))
        ps_ctx = ctx.enter_context(tc.tile_pool(name="ps_ctx", bufs=1, space="PSUM"))

        # ---------------- constants ----------------
        identf = singles.tile([128, 128], F32, name="identf")
        make_identity(nc, identf)
        ident = singles.tile([128, 128], F32R, name="ident")
        nc.vector.tensor_copy(out=ident, in_=identf)
        ones_f32 = singles.tile([1, NB], F32, name="ones_f32")
        nc.vector.memset(ones_f32, 1.0)
        ones_r = singles.tile([1, NB], F32R, name="ones_r")
        nc.vector.tensor_copy(out=ones_r, in_=ones_f32)
        zero_f32 = singles.tile([128, 2 * NB], F32, name="zero_f32")
        nc.vector.memset(zero_f32, 0.0)

        # Warmup transpose: consumes the identity's semaphore chain so later
        # transposes carry only one wait each (walrus limits LDW sync waits).
        warm_ps = ps_ev.tile([128, 128], F32R, tag="ev", name="warm_tp")
        nc.tensor.transpose(out=warm_ps, in_=ident, identity=ident)

        # ---------------- weights (SWDGE cast-DMA rounds f32 -> f32r) -------
        wi2h = singles.tile([128, 4, H], F32R, name="wi2h")
        nc.gpsimd.dma_start(out=wi2h, in_=d_Wi2h.ap().rearrange("(c p) h -> p c h", p=128))
        wh2h = singles.tile([128, 4, H], F32R, name="wh2h")
        nc.gpsimd.dma_start(out=wh2h, in_=d_Wh2h.ap().rearrange("(c p) h -> p c h", p=128))
        wsc = singles.tile([128, 4], F32R, name="wsc")
        nc.gpsimd.dma_start(out=wsc, in_=d_WscT.ap())
        bh2h_sb = singles.tile([1, H], F32R, name="bh2h_sb")
        nc.gpsimd.dma_start(out=bh2h_sb, in_=d_bh2h.ap())
        prev_hT = singles.tile([128, 4, NB], F32R, name="prev_hT")
        nc.gpsimd.dma_start(out=prev_hT, in_=d_prev_hT.ap().rearrange("(c p) b -> p c b", p=128))
        ohT_sb = singles.tile([NCLS, NB], F32R, name="ohT_sb")
        nc.gpsimd.dma_start(out=ohT_sb, in_=d_ohT.ap())
        prev_c_sb = singles.tile([NB, H], F32, name="prev_c_sb")
        nc.sync.dma_start(out=prev_c_sb, in_=d_prev_c.ap())

        # LSTM weights (loaded in ~1MB chunks, interleaved into the main loop
        # below to avoid head-of-line blocking; tiles declared here)
        wk_main = singles.tile([128, 4, ZD], F32R, name="wk_main")
        wk_tail = singles.tile([NCLS, ZD], F32R, name="wk_tail")
        uk = singles.tile([128, 4, ZD], F32R, name="uk")
        bk_sb = singles.tile([1, ZD], F32R, name="bk_sb")
        wk_ap = d_Wk_main.ap().rearrange("(c p) h -> p c h", p=128)
        uk_ap = d_Uk.ap().rearrange("(c p) h -> p c h", p=128)
        late_loads = (
            [(wk_main, wk_ap, kc) for kc in range(4)]
            + [(uk, uk_ap, kc) for kc in range(4)]
            + [(wk_tail, d_Wk_tail.ap(), None), (bk_sb, d_bk.ap(), None)]
        )

        # ---------------- h_projT = (prev_h @ Wh2h + bh2h).T ----------------
        hpT = singles.tile([128, 4, NB], F32, name="hpT")
        for hc in range(4):
            hp_ps = ps_g.tile([128, NB], F32, tag=f"g{hc}", name=f"hp_ps{hc}")
            for kc in range(4):
                nc.tensor.matmul(
                    hp_ps,
                    lhsT=wh2h[:, kc, hc * 128:(hc + 1) * 128],
                    rhs=prev_hT[:, kc, :],
                    start=(kc == 0),
                    stop=False,
                )
            nc.tensor.matmul(
                hp_ps,
                lhsT=bh2h_sb[0:1, hc * 128:(hc + 1) * 128],
                rhs=ones_r,
                start=False,
                stop=True,
            )
            nc.scalar.copy(out=hpT[:, hc, :], in_=hp_ps)

        # context accumulator: row b collects context[b]; all 64 matmuls
        # accumulate into this one PSUM tile (zero columns elsewhere).
        ctx_ps = ps_ctx.tile([NB, D], F32, name="ctx_ps")

        npairs = NB // 2
        for pr in range(npairs):
            bpair = (2 * pr, 2 * pr + 1)

            # interleave one ~1MB LSTM-weight chunk load per early pair
            if pr < len(late_loads):
                dst, src_ap, kc = late_loads[pr]
                if kc is None:
                    nc.gpsimd.dma_start(out=dst, in_=src_ap)
                else:
                    nc.gpsimd.dma_start(out=dst[:, kc, :], in_=src_ap[:, kc, :])

            nats = []
            for i, b in enumerate(bpair):
                nat = nat_pool.tile([128, 2, D], F32R, tag=f"nat{i}", name=f"nat{pr}_{i}")
                nc.gpsimd.dma_start(
                    out=nat, in_=bh[b].rearrange("(tc p) d -> p tc d", p=128)
                )
                nats.append(nat)

            # transpose batch_H into bHT [d, (b0 t | b1 t)], interleaved with
            # the G matmuls chunk-by-chunk so PE matmul activity stays dense
            # (keeps the HAM clock gate warm) and transposes overlap G.
            g_ps = [
                ps_g.tile([128, 2 * T], F32, tag=f"g{hc}", name=f"g_ps{pr}_{hc}")
                for hc in range(4)
            ]
            for dc in range(4):
                tp = ps_tp.tile([128, 2 * T], F32R, tag="tp", name=f"tp{pr}_{dc}")
                for i in range(2):
                    for tcb in range(2):
                        nc.tensor.transpose(
                            out=tp[:, i * T + tcb * 128: i * T + (tcb + 1) * 128],
                            in_=nats[i][:, tcb, dc * 128:(dc + 1) * 128],
                            identity=ident,
                        )
                bt = bht_pool.tile([128, 2 * T], F32R, tag=f"bht{dc}", name=f"bht{pr}_{dc}")
                nc.vector.tensor_copy(out=bt, in_=tp.bitcast(F32))
                for hc in range(4):
                    nc.tensor.matmul(
                        g_ps[hc],
                        lhsT=wi2h[:, dc, hc * 128:(hc + 1) * 128],
                        rhs=bt,
                        start=(dc == 0),
                        stop=(dc == 3),
                    )

            # tanh(G.T + h_projT[:,b]) per half
            tanhg = []
            for hc in range(4):
                tg = tg_pool.tile([128, 2 * T], F32R, tag=f"tg{hc}", name=f"tg{pr}_{hc}")
                for i, b in enumerate(bpair):
                    nc.scalar.activation(
                        out=tg[:, i * T:(i + 1) * T],
                        in_=g_ps[hc][:, i * T:(i + 1) * T],
                        func=AF.Tanh,
                        bias=hpT[:, hc, b:b + 1],
                        scale=1.0,
                    )
                tanhg.append(tg)

            # e = Wscore.T @ tanhG  -> [1, 2T]
            e_ps = ps_ev.tile([1, 2 * T], F32, tag="ev", name=f"e_ps{pr}")
            for hc in range(4):
                nc.tensor.matmul(
                    e_ps,
                    lhsT=wsc[:, hc:hc + 1],
                    rhs=tanhg[hc],
                    start=(hc == 0),
                    stop=(hc == 3),
                )

            # softmax + context per b
            for i, b in enumerate(bpair):
                esl = e_ps[0:1, i * T:(i + 1) * T]
                nm = sm_pool.tile([1, 1], F32, tag="nm", name=f"nm{b}")
                nc.vector.tensor_reduce(
                    out=nm, in_=esl, axis=AX.X, op=ALU.max, negate=True
                )
                p_sb = sm_pool.tile([1, T], F32, tag="p", name=f"p{b}")
                s_sb = sm_pool.tile([1, 1], F32, tag="s", name=f"s{b}")
                nc.scalar.activation(
                    out=p_sb, in_=esl, func=AF.Exp, bias=nm, scale=1.0,
                    accum_out=s_sb,
                )
                r_sb = sm_pool.tile([1, 1], F32, tag="r", name=f"r{b}")
                nc.vector.reciprocal(r_sb, s_sb)
                alpha_sb = sm_pool.tile([1, T], F32, tag="alpha", name=f"alpha{b}")
                nc.vector.tensor_scalar_mul(alpha_sb, p_sb, r_sb)
                nc.sync.dma_start(out=d_alpha.ap()[b:b + 1, :], in_=alpha_sb)

                # alphaT columns [t, 1] via fp32 matmul against ones[1,1]
                aT_ps = ps_ev.tile([128, 2], F32, tag="ev", name=f"aT_ps{b}")
                for tcb in range(2):
                    nc.tensor.matmul(
                        aT_ps[:, tcb:tcb + 1],
                        lhsT=alpha_sb[0:1, tcb * 128:(tcb + 1) * 128],
                        rhs=ones_f32[0:1, 0:1],
                        start=True,
                        stop=True,
                    )
                # zero-padded [128, 2, NB] stationary: column b = alphaT
                aT32 = sm_pool.tile([128, 2, NB], F32R, tag="aT32", name=f"aT32{b}")
                nc.vector.tensor_copy(out=aT32, in_=zero_f32)
                nc.vector.tensor_copy(out=aT32[:, :, b:b + 1], in_=aT_ps)
                for tcb in range(2):
                    nc.tensor.matmul(
                        ctx_ps,
                        lhsT=aT32[:, tcb, :],
                        rhs=nats[i][:, tcb, :],
                        start=(b == 0 and tcb == 0),
                        stop=(b == NB - 1 and tcb == 1),
                        skip_group_check=True,
                    )

        # ---------------- LSTM tail ----------------
        ctx_all = singles.tile([NB, D], F32R, name="ctx_all")
        nc.scalar.copy(out=ctx_all, in_=ctx_ps)
        ctxT = singles.tile([128, 4, NB], F32R, name="ctxT")
        for c in range(4):
            t_ps = ps_ev.tile([128, NB], F32R, tag="ev", name=f"ctxT_ps{c}")
            nc.tensor.transpose(
                out=t_ps, in_=ctx_all[:, c * 128:(c + 1) * 128],
                identity=ident[0:NB, 0:NB],
            )
            nc.scalar.copy(out=ctxT[:, c, :], in_=t_ps.bitcast(F32))

        gate_funcs = [AF.Sigmoid, AF.Sigmoid, AF.Tanh, AF.Sigmoid]
        gates = []
        for j in range(4):
            z_ps = ps_g.tile([NB, H], F32, tag=f"g{j}", name=f"z_ps{j}")
            for kc in range(4):
                nc.tensor.matmul(
                    z_ps, lhsT=ctxT[:, kc, :],
                    rhs=wk_main[:, kc, j * H:(j + 1) * H],
                    start=(kc == 0), stop=False,
                )
            nc.tensor.matmul(
                z_ps, lhsT=ohT_sb, rhs=wk_tail[:, j * H:(j + 1) * H],
                start=False, stop=False,
            )
            for kc in range(4):
                nc.tensor.matmul(
                    z_ps, lhsT=prev_hT[:, kc, :],
                    rhs=uk[:, kc, j * H:(j + 1) * H],
                    start=False, stop=False,
                )
            nc.tensor.matmul(
                z_ps, lhsT=ones_r, rhs=bk_sb[0:1, j * H:(j + 1) * H],
                start=False, stop=True,
            )
            g_sb = singles.tile([NB, H], F32, name=f"gate{j}")
            nc.scalar.activation(out=g_sb, in_=z_ps, func=gate_funcs[j], scale=1.0)
            gates.append(g_sb)

        t1 = singles.tile([NB, H], F32, name="t1")
        nc.vector.tensor_mul(t1, gates[1], prev_c_sb)
        t2 = singles.tile([NB, H], F32, name="t2")
        nc.vector.tensor_mul(t2, gates[0], gates[2])
        cnew = singles.tile([NB, H], F32, name="cnew")
        nc.vector.tensor_add(cnew, t1, t2)
        tanh_c = singles.tile([NB, H], F32, name="tanh_c")
        nc.scalar.activation(out=tanh_c, in_=cnew, func=AF.Tanh, scale=1.0)
        hnew = singles.tile([NB, H], F32, name="hnew")
        nc.vector.tensor_mul(hnew, gates[3], tanh_c)

        nc.sync.dma_start(out=d_h_new.ap(), in_=hnew)
        nc.sync.dma_start(out=d_c_new.ap(), in_=cnew)

    # Bacc post-passes (register allocation, wait-splitting for the 1-wait
    # hardware limit) run in finalize(); the bass2jax lowering serializes the
    # module as-is, so finalize must happen here.
    nc.finalize()
    return nc


_CACHED_NC = None


def kernel(prev_h, prev_c, batch_H, char_onehots, Wi2h, Wh2h, bh2h, Wscore, Wk, Uk, bk):
    global LAST_RESULT, _CACHED_NC
    from concourse.bass_utils import run_bass_kernel_spmd

    prev_h = np.ascontiguousarray(np.asarray(prev_h, dtype=np.float32))
    prev_c = np.ascontiguousarray(np.asarray(prev_c, dtype=np.float32))
    batch_H = np.ascontiguousarray(np.asarray(batch_H, dtype=np.float32))
    char_onehots = np.ascontiguousarray(np.asarray(char_onehots, dtype=np.float32))
    Wi2h = np.ascontiguousarray(np.asarray(Wi2h, dtype=np.float32))
    Wh2h = np.ascontiguousarray(np.asarray(Wh2h, dtype=np.float32))
    bh2h = np.asarray(bh2h, dtype=np.float32).reshape(1, H)
    Wscore = np.asarray(Wscore, dtype=np.float32)
    Wk = np.ascontiguousarray(np.asarray(Wk, dtype=np.float32))
    Uk = np.ascontiguousarray(np.asarray(Uk, dtype=np.float32))
    bk = np.asarray(bk, dtype=np.float32).reshape(1, ZD)

    # host-side prep of small tensors (layouts the kernel wants)
    WscT = np.ascontiguousarray(Wscore[:, 0].reshape(4, 128).T)       # [128, 4]
    Wk_main = np.ascontiguousarray(Wk[:D, :])
    Wk_tail = np.ascontiguousarray(Wk[D:, :])

    if _CACHED_NC is None:
        _CACHED_NC = build_bass()
    nc = _CACHED_NC

    in_maps = []
    for c in range(NCORES):
        sl = slice(c * NB, (c + 1) * NB)
        in_maps.append({
            "batch_H": np.ascontiguousarray(batch_H[sl]),
            "prev_hT": np.ascontiguousarray(prev_h[sl].T),            # [H, NB]
            "prev_c": np.ascontiguousarray(prev_c[sl]),
            "ohT": np.ascontiguousarray(char_onehots[sl].T),          # [NCLS, NB]
            "Wi2h": Wi2h,
            "Wh2h": Wh2h,
            "bh2h": bh2h,
            "WscT": WscT,
            "Wk_main": Wk_main,
            "Wk_tail": Wk_tail,
            "Uk": Uk,
            "bk": bk,
        })

    trace = bool(int(os.environ.get("BASS_KERNEL_TRACE", "0")))
    res = run_bass_kernel_spmd(nc, in_maps, core_ids=list(range(NCORES)), trace=trace)
    LAST_RESULT = res

    outs = res.results
    h_new = np.concatenate([np.asarray(o["h_new"]) for o in outs], axis=0)
    c_new = np.concatenate([np.asarray(o["c_new"]) for o in outs], axis=0)
    alpha = np.concatenate([np.asarray(o["alpha"]) for o in outs], axis=0)
    return h_new, c_new, alpha[:, :, None]


# revision 13
# speedup vs baseline: 1.2636x; 1.2636x over previous
"""Trainium2 Bass kernel for nn_AttentionCell (Bahdanau attention + LSTM cell).

Full shapes: B=256, T=256, D_ENC=512, H=512, NUM_CLASSES=96.
Sharding: data-parallel over batch across 8 NeuronCores (32 rows each),
all parameters replicated. batch_H is shipped host-pretransposed per core as
bhT[pair, dchunk, 128, 2, T] so no on-chip transposes are needed and the PE
runs dense float32r matmuls (1 cycle/row, ~1e-4 rounding).

Per-core pipeline (b processed in pairs):
  - DMA bhT tiles [128d, 2b, 256t] with SWDGE cast f32 -> f32r
  - G.T[h, t] = Wi2h.T @ bhT on PE (f32r, N=512)
  - tanh(G.T + h_projT[:, b]) fused on ScalarE (bias per-partition here)
  - e[1, 2T] = Wscore.T @ tanhG.T on PE
  - softmax over t on partition 0: DVE max(negate) -> ACT exp(bias=-max,
    accum_out=sum) -> DVE reciprocal -> DVE tensor_scalar mul
  - alpha broadcast across partitions via 1 PE matmul (ones column), then
    context.T columns via fused DVE tensor_tensor_reduce over the bhT tiles
  - LSTM: z = x @ Wk + prev_h @ Uk + bk on PE (f32r) with host-pretransposed
    prev_h.T / onehots.T; gates on ScalarE/VectorE in fp32
"""

import os
import sys
from contextlib import ExitStack

import numpy as np

sys.path.insert(0, "/opt/trn_rl_repo")

import concourse.bass as bass  # noqa: E402,F401
import concourse.tile as tile  # noqa: E402
from concourse import bacc, dve_ops, mybir  # noqa: E402

F32 = mybir.dt.float32
F32R = mybir.dt.float32r
AF = mybir.ActivationFunctionType
ALU = mybir.AluOpType
AX = mybir.AxisListType

# Problem constants (hardcoded; kernel.py must be self-contained)
B, T, D, H, NCLS = 256, 256, 512, 512, 96
NCORES = 8
NB = B // NCORES           # 32 batch rows per core
NPAIRS = NB // 2
ZD = 4 * H                 # 2048 LSTM gate width

LAST_RESULT = None         # stashes BassKernelResults for test harness


def build_bass():
    nc = bacc.Bacc("TRN2")

    # ---------------- DRAM I/O (per-core shard shapes) ----------------
    # bhT[pr, dc, p, i, t] = batch_H[2*pr+i, t, 128*dc+p]
    d_bhT = nc.dram_tensor("bhT", [NPAIRS, 4, 128, 2, T], F32, kind="ExternalInput")
    d_prev_hT = nc.dram_tensor("prev_hT", [H, NB], F32, kind="ExternalInput")
    d_prev_c = nc.dram_tensor("prev_c", [NB, H], F32, kind="ExternalInput")
    d_ohT = nc.dram_tensor("ohT", [NCLS, NB], F32, kind="ExternalInput")
    d_Wi2h = nc.dram_tensor("Wi2h", [D, H], F32, kind="ExternalInput")
    d_Wh2h = nc.dram_tensor("Wh2h", [H, H], F32, kind="ExternalInput")
    d_bh2h = nc.dram_tensor("bh2h", [1, H], F32, kind="ExternalInput")
    d_WscT = nc.dram_tensor("WscT", [128, 4], F32, kind="ExternalInput")
    d_Wk_main = nc.dram_tensor("Wk_main", [D, ZD], F32, kind="ExternalInput")
    d_Wk_tail = nc.dram_tensor("Wk_tail", [NCLS, ZD], F32, kind="ExternalInput")
    d_Uk = nc.dram_tensor("Uk", [H, ZD], F32, kind="ExternalInput")
    d_bk = nc.dram_tensor("bk", [1, ZD], F32, kind="ExternalInput")

    d_h_new = nc.dram_tensor("h_new", [NB, H], F32, kind="ExternalOutput")
    d_c_new = nc.dram_tensor("c_new", [NB, H], F32, kind="ExternalOutput")
    d_alpha = nc.dram_tensor("alpha", [NB, T], F32, kind="ExternalOutput")

    bhT_ap = d_bhT.ap()

    with tile.TileContext(nc) as tc, ExitStack() as ctx:
        singles = ctx.enter_context(tc.tile_pool(name="singles", bufs=1))
        bht_pool = ctx.enter_context(tc.tile_pool(name="bht", bufs=2))
        tg_pool = ctx.enter_context(tc.tile_pool(name="tg", bufs=2))
        sm_pool = ctx.enter_context(tc.tile_pool(name="sm", bufs=3))
        ps_g = ctx.enter_context(tc.tile_pool(name="ps_g", bufs=1, space="PSUM"))
        ps_ev = ctx.enter_context(tc.tile_pool(name="ps_ev", bufs=3, space="PSUM"))

        # ---------------- constants ----------------
        ones_f32 = singles.tile([1, NB], F32, name="ones_f32")
        nc.vector.memset(ones_f32, 1.0)
        ones_r = singles.tile([1, NB], F32R, name="ones_r")
        nc.vector.tensor_copy(out=ones_r, in_=ones_f32)
        onesc_f32 = singles.tile([1, 128], F32, name="onesc_f32")
        nc.vector.memset(onesc_f32, 1.0)
        onesc_r = singles.tile([1, 128], F32R, name="onesc_r")
        nc.vector.tensor_copy(out=onesc_r, in_=onesc_f32)

        # ---------------- weights (SWDGE cast-DMA rounds f32 -> f32r) -------
        wi2h = singles.tile([128, 4, H], F32R, name="wi2h")
        nc.gpsimd.dma_start(out=wi2h, in_=d_Wi2h.ap().rearrange("(c p) h -> p c h", p=128))
        wh2h = singles.tile([128, 4, H], F32R, name="wh2h")
        nc.gpsimd.dma_start(out=wh2h, in_=d_Wh2h.ap().rearrange("(c p) h -> p c h", p=128))
        wsc = singles.tile([128, 4], F32R, name="wsc")
        nc.gpsimd.dma_start(out=wsc, in_=d_WscT.ap())
        bh2h_sb = singles.tile([1, H], F32R, name="bh2h_sb")
        nc.gpsimd.dma_start(out=bh2h_sb, in_=d_bh2h.ap())
        prev_hT = singles.tile([128, 4, NB], F32R, name="prev_hT")
        nc.gpsimd.dma_start(out=prev_hT, in_=d_prev_hT.ap().rearrange("(c p) b -> p c b", p=128))
        ohT_sb = singles.tile([NCLS, NB], F32R, name="ohT_sb")
        nc.gpsimd.dma_start(out=ohT_sb, in_=d_ohT.ap())
        prev_c_sb = singles.tile([NB, H], F32, name="prev_c_sb")
        nc.sync.dma_start(out=prev_c_sb, in_=d_prev_c.ap())

        # LSTM weights (loaded in ~1MB chunks, interleaved into the main loop
        # below to avoid head-of-line blocking; tiles declared here)
        wk_main = singles.tile([128, 4, ZD], F32R, name="wk_main")
        wk_tail = singles.tile([NCLS, ZD], F32R, name="wk_tail")
        uk = singles.tile([128, 4, ZD], F32R, name="uk")
        bk_sb = singles.tile([1, ZD], F32R, name="bk_sb")
        wk_ap = d_Wk_main.ap().rearrange("(c p) h -> p c h", p=128)
        uk_ap = d_Uk.ap().rearrange("(c p) h -> p c h", p=128)
        late_loads = (
            [(wk_main, wk_ap, kc) for kc in range(4)]
            + [(uk, uk_ap, kc) for kc in range(4)]
            + [(wk_tail, d_Wk_tail.ap(), None), (bk_sb, d_bk.ap(), None)]
        )

        # ---------------- h_projT = (prev_h @ Wh2h + bh2h).T ----------------
        hpT = singles.tile([128, 4, NB], F32, name="hpT")
        for hc in range(4):
            hp_ps = ps_g.tile([128, NB], F32, tag=f"g{hc}", name=f"hp_ps{hc}")
            for kc in range(4):
                nc.tensor.matmul(
                    hp_ps,
                    lhsT=wh2h[:, kc, hc * 128:(hc + 1) * 128],
                    rhs=prev_hT[:, kc, :],
                    start=(kc == 0),
                    stop=False,
                )
            nc.tensor.matmul(
                hp_ps,
                lhsT=bh2h_sb[0:1, hc * 128:(hc + 1) * 128],
                rhs=ones_r,
                start=False,
                stop=True,
            )
            nc.scalar.copy(out=hpT[:, hc, :], in_=hp_ps)

        # context.T accumulator in SBUF: column b of chunk dc = context[b][dc]
        ctxT_f = singles.tile([128, 4, NB], F32, name="ctxT_f")

        for pr in range(NPAIRS):
            bpair = (2 * pr, 2 * pr + 1)

            # interleave one ~1MB LSTM-weight chunk load per early pair
            if pr < len(late_loads):
                dst, src_ap, kc = late_loads[pr]
                if kc is None:
                    nc.gpsimd.dma_start(out=dst, in_=src_ap)
                else:
                    nc.gpsimd.dma_start(out=dst[:, kc, :], in_=src_ap[:, kc, :])

            # load transposed batch_H tiles [128 d, (b0 t | b1 t)]
            bht = []
            for dc in range(4):
                bt = bht_pool.tile([128, 2, T], F32R, tag=f"bht{dc}", name=f"bht{pr}_{dc}")
                nc.gpsimd.dma_start(out=bt, in_=bhT_ap[pr, dc])
                bht.append(bt)

            # G.T = Wi2h.T @ bhT ; tanh(G.T + h_projT[:,b]) per half
            g_ps = [
                ps_g.tile([128, 2 * T], F32, tag=f"g{hc}", name=f"g_ps{pr}_{hc}")
                for hc in range(4)
            ]
            for dc in range(4):
                for hc in range(4):
                    nc.tensor.matmul(
                        g_ps[hc],
                        lhsT=wi2h[:, dc, hc * 128:(hc + 1) * 128],
                        rhs=bht[dc],
                        start=(dc == 0),
                        stop=(dc == 3),
                    )
            tanhg = []
            for hc in range(4):
                tg = tg_pool.tile([128, 2 * T], F32R, tag=f"tg{hc}", name=f"tg{pr}_{hc}")
                for i, b in enumerate(bpair):
                    nc.scalar.activation(
                        out=tg[:, i * T:(i + 1) * T],
                        in_=g_ps[hc][:, i * T:(i + 1) * T],
                        func=AF.Tanh,
                        bias=hpT[:, hc, b:b + 1],
                        scale=1.0,
                    )
                tanhg.append(tg)

            # e = Wscore.T @ tanhG  -> [1, 2T]
            e_ps = ps_ev.tile([1, 2 * T], F32, tag="ev", name=f"e_ps{pr}")
            for hc in range(4):
                nc.tensor.matmul(
                    e_ps,
                    lhsT=wsc[:, hc:hc + 1],
                    rhs=tanhg[hc],
                    start=(hc == 0),
                    stop=(hc == 3),
                )

            # softmax + context per b
            for i, b in enumerate(bpair):
                esl = e_ps[0:1, i * T:(i + 1) * T]
                nm = sm_pool.tile([1, 1], F32, tag="nm", name=f"nm{b}")
                nc.vector.tensor_reduce(
                    out=nm, in_=esl, axis=AX.X, op=ALU.max, negate=True
                )
                p_sb = sm_pool.tile([1, T], F32, tag="p", name=f"p{b}")
                s_sb = sm_pool.tile([1, 1], F32, tag="s", name=f"s{b}")
                nc.scalar.activation(
                    out=p_sb, in_=esl, func=AF.Exp, bias=nm, scale=1.0,
                    accum_out=s_sb,
                )
                r_sb = sm_pool.tile([1, 1], F32, tag="r", name=f"r{b}")
                nc.vector.reciprocal(r_sb, s_sb)
                alpha_sb = sm_pool.tile([1, T], F32, tag="alpha", name=f"alpha{b}")
                nc.vector.tensor_scalar_mul(alpha_sb, p_sb, r_sb)
                nc.sync.dma_start(out=d_alpha.ap()[b:b + 1, :], in_=alpha_sb)
                alpha_r = sm_pool.tile([1, T], F32R, tag="alphar", name=f"alphar{b}")
                nc.vector.tensor_copy(out=alpha_r, in_=alpha_sb)

                # broadcast alpha across 128 partitions via PE (ones column)
                bc_ps = ps_ev.tile([128, T], F32, tag="ev", name=f"bc_ps{b}")
                nc.tensor.matmul(
                    bc_ps, lhsT=onesc_r, rhs=alpha_r, start=True, stop=True,
                )
                bc_sb = sm_pool.tile([128, T], F32, tag="bc", name=f"bc{b}")
                nc.vector.tensor_copy(out=bc_sb, in_=bc_ps)

                # context.T[dc][:, b] = sum_t bhT[dc][:, i, t] * alpha[t]
                for dc in range(4):
                    scr = sm_pool.tile([128, T], F32, tag="scr", name=f"scr{b}_{dc}")
                    # custom-DVE TTR: out = in0*in1*s1; accum_out = s0 + sum(out)
                    # (the ISA-opcode tensor_tensor_reduce crashes this ucode)
                    nc.vector._custom_dve(
                        dve_ops.TENSOR_TENSOR_REDUCE,
                        out=scr,
                        in0=bht[dc][:, i, :].bitcast(F32),
                        in1=bc_sb,
                        s0=0.0,
                        s1=1.0,
                        accum_out=ctxT_f[:, dc, b:b + 1],
                    )

        # ---------------- LSTM tail ----------------
        ctxT = singles.tile([128, 4, NB], F32R, name="ctxT")
        nc.vector.tensor_copy(out=ctxT, in_=ctxT_f)

        gate_funcs = [AF.Sigmoid, AF.Sigmoid, AF.Tanh, AF.Sigmoid]
        gates = []
        for j in range(4):
            z_ps = ps_g.tile([NB, H], F32, tag=f"g{j}", name=f"z_ps{j}")
            for kc in range(4):
                nc.tensor.matmul(
                    z_ps, lhsT=ctxT[:, kc, :],
                    rhs=wk_main[:, kc, j * H:(j + 1) * H],
                    start=(kc == 0), stop=False,
                )
            nc.tensor.matmul(
                z_ps, lhsT=ohT_sb, rhs=wk_tail[:, j * H:(j + 1) * H],
                start=False, stop=False,
            )
            for kc in range(4):
                nc.tensor.matmul(
                    z_ps, lhsT=prev_hT[:, kc, :],
                    rhs=uk[:, kc, j * H:(j + 1) * H],
                    start=False, stop=False,
                )
            nc.tensor.matmul(
                z_ps, lhsT=ones_r, rhs=bk_sb[0:1, j * H:(j + 1) * H],
                start=False, stop=True,
            )
            g_sb = singles.tile([NB, H], F32, name=f"gate{j}")
            nc.scalar.activation(out=g_sb, in_=z_ps, func=gate_funcs[j], scale=1.0)
            gates.append(g_sb)

        t1 = singles.tile([NB, H], F32, name="t1")
        nc.vector.tensor_mul(t1, gates[1], prev_c_sb)
        t2 = singles.tile([NB, H], F32, name="t2")
        nc.vector.tensor_mul(t2, gates[0], gates[2])
        cnew = singles.tile([NB, H], F32, name="cnew")
        nc.vector.tensor_add(cnew, t1, t2)
        tanh_c = singles.tile([NB, H], F32, name="tanh_c")
        nc.scalar.activation(out=tanh_c, in_=cnew, func=AF.Tanh, scale=1.0)
        hnew = singles.tile([NB, H], F32, name="hnew")
        nc.vector.tensor_mul(hnew, gates[3], tanh_c)

        nc.sync.dma_start(out=d_h_new.ap(), in_=hnew)
        nc.sync.dma_start(out=d_c_new.ap(), in_=cnew)

    # Bacc post-passes (register allocation, wait-splitting for the 1-wait
    # hardware limit) run in finalize(); the bass2jax lowering serializes the
    # module as-is, so finalize must happen here.
    nc.finalize()
    return nc


_CACHED_NC = None


def kernel(prev_h, prev_c, batch_H, char_onehots, Wi2h, Wh2h, bh2h, Wscore, Wk, Uk, bk):
    global LAST_RESULT, _CACHED_NC
    from concourse.bass_utils import run_bass_kernel_spmd

    prev_h = np.ascontiguousarray(np.asarray(prev_h, dtype=np.float32))
    prev_c = np.ascontiguousarray(np.asarray(prev_c, dtype=np.float32))
    batch_H = np.ascontiguousarray(np.asarray(batch_H, dtype=np.float32))
    char_onehots = np.ascontiguousarray(np.asarray(char_onehots, dtype=np.float32))
    Wi2h = np.ascontiguousarray(np.asarray(Wi2h, dtype=np.float32))
    Wh2h = np.ascontiguousarray(np.asarray(Wh2h, dtype=np.float32))
    bh2h = np.asarray(bh2h, dtype=np.float32).reshape(1, H)
    Wscore = np.asarray(Wscore, dtype=np.float32)
    Wk = np.ascontiguousarray(np.asarray(Wk, dtype=np.float32))
    Uk = np.ascontiguousarray(np.asarray(Uk, dtype=np.float32))
    bk = np.asarray(bk, dtype=np.float32).reshape(1, ZD)

    # host-side prep (layouts the kernel wants)
    WscT = np.ascontiguousarray(Wscore[:, 0].reshape(4, 128).T)       # [128, 4]
    Wk_main = np.ascontiguousarray(Wk[:D, :])
    Wk_tail = np.ascontiguousarray(Wk[D:, :])

    if _CACHED_NC is None:
        _CACHED_NC = build_bass()
    nc = _CACHED_NC

    in_maps = []
    for c in range(NCORES):
        sl = slice(c * NB, (c + 1) * NB)
        # bhT[pr, dc, p, i, t] = batch_H[2*pr+i, t, 128*dc+p]
        shard = batch_H[sl]                                   # [NB, T, D]
        bhT = np.ascontiguousarray(
            shard.reshape(NPAIRS, 2, T, 4, 128).transpose(0, 3, 4, 1, 2)
        )
        in_maps.append({
            "bhT": bhT,
            "prev_hT": np.ascontiguousarray(prev_h[sl].T),            # [H, NB]
            "prev_c": np.ascontiguousarray(prev_c[sl]),
            "ohT": np.ascontiguousarray(char_onehots[sl].T),          # [NCLS, NB]
            "Wi2h": Wi2h,
            "Wh2h": Wh2h,
            "bh2h": bh2h,
            "WscT": WscT,
            "Wk_main": Wk_main,
            "Wk_tail": Wk_tail,
            "Uk": Uk,
            "bk": bk,
        })

    trace = bool(int(os.environ.get("BASS_KERNEL_TRACE", "0")))
    res = run_bass_kernel_spmd(nc, in_maps, core_ids=list(range(NCORES)), trace=trace)
    LAST_RESULT = res

    outs = res.results
    h_new = np.concatenate([np.asarray(o["h_new"]) for o in outs], axis=0)
    c_new = np.concatenate([np.asarray(o["c_new"]) for o in outs], axis=0)
    alpha = np.concatenate([np.asarray(o["alpha"]) for o in outs], axis=0)
    return h_new, c_new, alpha[:, :, None]
